# revision 4
# baseline (speedup 1.0000x reference)
"""Multi-head attention (B=4, L=2048, C=1024, H=16, HD=64) on 8 NeuronCores.

Sharding: tensor-parallel over heads - 2 heads per core. Each core computes
its heads' QKV projection, attention, and a partial output projection over
its 128 ctx channels; the host sums the 8 partial outputs (b_out added on
host). All HW-side matmul operands are bf16 (measured rms rel err 2.4e-3,
well under the 2e-2 gate); PSUM accumulation stays fp32.

Key measured facts driving the layout (TRN2, from ntff traces):
  - PE streams 1 col/cycle @2.4GHz for bf16/f32r/fp8 alike; f32r pays an
    extra 128-cycle self-load per matmul (ldweights can't pipeline fp32),
    so bf16 operands save ~53ns on every 512-col matmul.
  - fp8 DoubleRow only pays off at K>=256 per instruction; scores (K=128)
    and outproj (K=128) get nothing, and fp8 anywhere on the qkv/attnV
    path pushes rms err past 1e-2. So: no fp8.
  - ACT exp [128,1024] from PSUM ~1.1us; exp+identity+copy share one ACT
    table (no swap cost).

Math notes:
  - softmax((q+bq).(k+bk)) == softmax((q+bq).k) per query: the (q+bq).bk
    term is constant across keys. So the k bias is dropped entirely and
    only bq is applied (free, rides the PSUM->SBUF drain).
  - Softmax skips the max-subtraction (scores ~N(0,1/9): exp is safe) and
    normalizes after the ctx matmul using ones-columns in the v tiles
    (rowsum rides as PSUM partition 64 of the ctx accumulation); 1/Z via
    DVE reciprocal + gpsimd partition broadcast (no ACT table swaps).
  - The per-head q chunks are zero-padded to the 128-channel frame (done
    once per tile instance via gpsimd memset) so the scores matmul always
    runs K=128 (PE shape switches measured ~0.9us).
"""

import numpy as np
import ml_dtypes

import concourse.bass as bass
import concourse.mybir as mybir
import concourse.tile as tile
from concourse import bacc
from concourse.bass_utils import run_bass_kernel_spmd

B, L, C, H, HD = 4, 2048, 1024, 16, 64
NCORES = 8
HPC = H // NCORES  # heads per core = 2
F32 = mybir.dt.float32
BF16 = mybir.dt.bfloat16
EXP = mybir.ActivationFunctionType.Exp

LCHUNK = 512          # token chunk for moving operands
NLC = L // LCHUNK     # 4
NKT = L // 128        # 16 k tiles per sequence
NCT = C // 128        # 8 contraction tiles for the projections


def build_kernel():
    nc = bacc.Bacc("TRN2", target_bir_lowering=False, debug=False,
                   num_devices=NCORES)

    xT = nc.dram_tensor("xT", [B, C, L], BF16, kind="ExternalInput")
    # wqkv[ci, j] = [128 c, 128 f] tile; j in (0=q both heads, 1=k, 2=v)
    wqkv = nc.dram_tensor("wqkv", [NCT, 3, 128, 128], BF16, kind="ExternalInput")
    bq_d = nc.dram_tensor("bq", [128, 1], F32, kind="ExternalInput")
    bv_d = nc.dram_tensor("bv", [128, 1], F32, kind="ExternalInput")
    # wo2: [128 c(2 heads), 1024 o]
    wo2 = nc.dram_tensor("wo2", [128, C], BF16, kind="ExternalInput")
    identb_d = nc.dram_tensor("identb_d", [128, 128], BF16, kind="ExternalInput")
    out = nc.dram_tensor("out", [B * L, C], BF16, kind="ExternalOutput")

    with tile.TileContext(nc) as tc:
        kernel_body(nc, tc, xT, wqkv, bq_d, bv_d, wo2, identb_d, out)
    nc.compile()
    return nc


def kernel_body(nc, tc, xT, wqkv, bq_d, bv_d, wo2, identb_d, out):
    from contextlib import ExitStack
    ctx = ExitStack()
    with ctx:
        consts = ctx.enter_context(tc.tile_pool(name="consts", bufs=1))
        xpool = ctx.enter_context(tc.tile_pool(name="xpool", bufs=16))
        qkvpool = ctx.enter_context(tc.tile_pool(name="qkvpool", bufs=2))
        vppool = ctx.enter_context(tc.tile_pool(name="vppool", bufs=24))
        epool = ctx.enter_context(tc.tile_pool(name="epool", bufs=14))
        cpool = ctx.enter_context(tc.tile_pool(name="cpool", bufs=2))
        spool = ctx.enter_context(tc.tile_pool(name="spool", bufs=2))
        opool = ctx.enter_context(tc.tile_pool(name="opool", bufs=4))
        # PSUM banks: s-tiles 2x2 + cacc 2 + general 2 = 8
        spsum = ctx.enter_context(tc.tile_pool(name="spsum", bufs=2,
                                               space="PSUM"))
        cpsum = ctx.enter_context(tc.tile_pool(name="cpsum", bufs=2,
                                               space="PSUM"))
        gpsum = ctx.enter_context(tc.tile_pool(name="gpsum", bufs=2,
                                               space="PSUM"))

        # ---- constants ----
        w_tiles = []
        for ci in range(NCT):
            row = []
            for j in range(3):
                t = consts.tile([128, 128], BF16, tag=f"w{ci}_{j}")
                nc.sync.dma_start(out=t, in_=wqkv[ci, j])
                row.append(t)
            w_tiles.append(row)
        bq_t = consts.tile([128, 1], F32, tag="bq_t")
        nc.sync.dma_start(out=bq_t, in_=bq_d[:])
        bv_t = consts.tile([128, 1], F32, tag="bv_t")
        nc.sync.dma_start(out=bv_t, in_=bv_d[:])
        wo_t = consts.tile([128, C], BF16, tag="wo_t")
        nc.sync.dma_start(out=wo_t, in_=wo2[:])
        identb = consts.tile([128, 128], BF16, tag="identb")
        nc.sync.dma_start(out=identb, in_=identb_d[:])

        # ---- phase helpers (emitted in software-pipelined order below) ----
        def emit_qkv_loads(b, pair):
            # 8 x tiles of [128, 1024] covering token range pair*1024..+1024
            ls = bass.ts(pair, 2 * LCHUNK)
            xts = []
            for ci in range(NCT):
                xt = xpool.tile([128, 2 * LCHUNK], BF16, tag="xt", name="xt")
                nc.sync.dma_start(out=xt, in_=xT[b, bass.ts(ci, 128), ls])
                xts.append(xt)
            return xts

        def emit_qkv_block(b, lc, qkvT, xts):
            ls = bass.ts(lc, LCHUNK)
            xs = bass.ts(lc % 2, LCHUNK)
            for j in range(3):
                p = gpsum.tile([128, LCHUNK], F32, tag="gpb", name="p")
                for ci in range(NCT):
                    nc.tensor.matmul(p, w_tiles[ci][j][:], xts[ci][:, xs],
                                     start=(ci == 0), stop=(ci == NCT - 1))
                if j == 0:
                    # q: split per head into the zero-padded 128-channel
                    # frame (other head's rows stay zero from the per-tile
                    # memset); bias bq rides the drain.
                    q0p, q1p = qkvT[0]
                    nc.vector.tensor_scalar_add(q0p[0:HD, ls], p[0:HD, :],
                                                bq_t[0:HD])
                    nc.vector.tensor_scalar_add(q1p[HD:128, ls], p[HD:128, :],
                                                bq_t[HD:128])
                elif j == 1:
                    # k: no bias (softmax-invariant, see module docstring)
                    nc.vector.tensor_copy(qkvT[1][:, ls], p)
                else:
                    nc.vector.tensor_scalar_add(qkvT[2][:, ls], p, bv_t)

        def emit_vplus(qkvT):
            # v -> token-major bf16 tiles [128 l, v_h0 | 1 | v_h1 | 1]
            vplus = []
            for t in range(NKT):
                tp = gpsum.tile([128, 128], BF16, tag="gpb", name="tp")
                nc.tensor.transpose(tp, qkvT[2][:, bass.ts(t, 128)], identb[:])
                vp = vppool.tile([128, 2 * HD + 2], BF16, tag="vp", name="vp")
                nc.vector.tensor_copy(vp[:, 0:HD], tp[:, 0:HD])
                nc.vector.tensor_copy(vp[:, HD + 1:2 * HD + 1], tp[:, HD:2 * HD])
                nc.gpsimd.memset(vp[:, HD:HD + 1], 1.0)
                nc.gpsimd.memset(vp[:, 2 * HD + 1:2 * HD + 2], 1.0)
                vplus.append(vp)
            return vplus

        def emit_attn_core(h, qc, qkvT, vplus, ctxT2):
            # one head, one 1024-wide q chunk: scores -> exp -> ctx matmul,
            # then inline normalize (reciprocal + broadcast + fused drain)
            vsl = slice(h * (HD + 1), (h + 1) * (HD + 1))
            q0 = qc * 1024
            caccs = [cpsum.tile([HD + 1, LCHUNK], F32, tag="cpb",
                                name=f"cacc{half}")
                     for half in range(2)]
            evec = []
            qhp = qkvT[0][h]
            for i in range(NKT):
                s = spsum.tile([128, 2 * LCHUNK], F32, tag="spb", name="s")
                for half in range(2):
                    nc.tensor.matmul(
                        s[:, bass.ts(half, LCHUNK)],
                        qkvT[1][:, bass.ts(i, 128)],
                        qhp[:, bass.ds(q0 + half * LCHUNK, LCHUNK)],
                        start=True, stop=True)
                e = epool.tile([128, 2 * LCHUNK], BF16, tag="e", name="e")
                nc.scalar.activation(e, s, EXP, scale=0.125)
                evec.append(e)
            for i in range(NKT):
                for half in range(2):
                    nc.tensor.matmul(
                        caccs[half],
                        vplus[i][:, vsl],
                        evec[i][:, bass.ts(half, LCHUNK)],
                        start=(i == 0), stop=(i == NKT - 1))
            for half in range(2):
                cacc = caccs[half]
                qs = bass.ds(q0 + half * LCHUNK, LCHUNK)
                zr = spool.tile([1, LCHUNK], F32, tag="zr", name="zr", bufs=4)
                nc.vector.reciprocal(zr[0:1, :], cacc[HD:HD + 1, :])
                zs = spool.tile([HD, LCHUNK], F32, tag="zs", name="zs", bufs=3)
                nc.gpsimd.partition_broadcast(zs[0:HD, :], zr[0:1, :])
                nc.vector.tensor_mul(ctxT2[h * HD:h * HD + HD, qs],
                                     cacc[0:HD, :], zs)

        def emit_outproj(b, ctxT2, trange):
            for t in trange:
                rows = bass.ds(b * L + t * 128, 128)
                ot = opool.tile([128, C], BF16, tag="ot", name="ot")
                for oc in range(C // 512):
                    os_ = bass.ts(oc, 512)
                    o = gpsum.tile([128, 512], F32, tag="gpb", name="o")
                    nc.tensor.matmul(o, ctxT2[:, bass.ts(t, 128)],
                                     wo_t[:, os_], start=True, stop=True)
                    nc.vector.tensor_copy(ot[:, os_], o)
                nc.sync.dma_start(out=out[rows, :], in_=ot)

        # ---- software-pipelined emission ----
        # Interleave next batch's qkv blocks between attention chunks so the
        # PE's in-order queue always has dense, ready work behind any stall.
        def new_qkvT():
            qp = tuple(qkvpool.tile([128, L], BF16, tag=f"q{h}p",
                                    name=f"q{h}p") for h in range(HPC))
            kc = qkvpool.tile([128, L], BF16, tag="kc", name="kc")
            vc = qkvpool.tile([128, L], BF16, tag="vc", name="vc")
            # zero the other head's rows once per tile instance; the q
            # drains only write their own head's 64 rows.
            nc.gpsimd.memset(qp[0][HD:128, :], 0.0)
            nc.gpsimd.memset(qp[1][0:HD, :], 0.0)
            return [qp, kc, vc]

        qkvT = new_qkvT()
        loads = {0: emit_qkv_loads(0, 0), 1: emit_qkv_loads(0, 1)}
        for lc in range(NLC):
            emit_qkv_block(0, lc, qkvT, loads[lc // 2])
        vplus = emit_vplus(qkvT)
        for b in range(B):
            ctxT2 = cpool.tile([128, L], BF16, tag="ctxT2", name="ctxT2")
            nxt = new_qkvT() if b + 1 < B else None
            # column-half order: both heads of q-chunk 0, then q-chunk 1, so
            # outproj l-tiles can start after the first half completes
            chunks = [(0, 0), (1, 0), (0, 1), (1, 1)]
            loads = {}
            if nxt is not None:
                loads[0] = emit_qkv_loads(b + 1, 0)
            for k, (h, qc) in enumerate(chunks):
                emit_attn_core(h, qc, qkvT, vplus, ctxT2)
                if nxt is not None:
                    if k == 1:
                        loads[1] = emit_qkv_loads(b + 1, 1)
                    emit_qkv_block(b + 1, k, nxt, loads[k // 2])
                if k == 1:
                    emit_outproj(b, ctxT2, range(0, NKT // 2))
            if nxt is not None:
                nxt_vplus = emit_vplus(nxt)
            emit_outproj(b, ctxT2, range(NKT // 2, NKT))
            if nxt is not None:
                qkvT, vplus = nxt, nxt_vplus


_NC_CACHE = None


def get_nc():
    global _NC_CACHE
    if _NC_CACHE is None:
        _NC_CACHE = build_kernel()
    return _NC_CACHE


def prepare_in_maps(x, W_qkv, b_qkv, W_out, b_out):
    x = np.asarray(x, np.float32)
    W_qkv = np.asarray(W_qkv, np.float32)
    b_qkv = np.asarray(b_qkv, np.float32)
    W_out = np.asarray(W_out, np.float32)

    xT = np.ascontiguousarray(x.transpose(0, 2, 1)).astype(ml_dtypes.bfloat16)

    in_maps = []
    for core in range(NCORES):
        h0 = HPC * core
        # per-head channel rows in W_qkv: q = h*192..+64, k = +64, v = +128
        qrows = [np.arange(h * 192, h * 192 + 64) for h in (h0, h0 + 1)]
        krows = [q + 64 for q in qrows]
        vrows = [q + 128 for q in qrows]
        fq = np.concatenate(qrows)
        fk = np.concatenate(krows)
        fv = np.concatenate(vrows)
        # wqkv tiles: [ci, j, 128 c, 128 f]
        wt = np.empty((NCT, 3, 128, 128), ml_dtypes.bfloat16)
        for j, rows in enumerate((fq, fk, fv)):
            wT = np.ascontiguousarray(W_qkv[rows].T)  # [1024 c, 128 f]
            wt[:, j] = wT.reshape(NCT, 128, 128).astype(ml_dtypes.bfloat16)
        # wo2 = [128 c, 1024 o]: rows 0:64 h0 ctx channels, 64:128 h1
        wo2 = np.concatenate([
            np.ascontiguousarray(W_out[:, (h0 + h) * HD:(h0 + h + 1) * HD].T)
            for h in range(HPC)
        ], axis=0)
        in_maps.append({
            "xT": xT,
            "wqkv": wt,
            "bq": np.ascontiguousarray(b_qkv[fq][:, None], np.float32),
            "bv": np.ascontiguousarray(b_qkv[fv][:, None], np.float32),
            "wo2": np.ascontiguousarray(wo2).astype(ml_dtypes.bfloat16),
            "identb_d": np.eye(128, dtype=ml_dtypes.bfloat16),
        })
    return in_maps


def kernel(x, W_qkv, b_qkv, W_out, b_out):
    in_maps = prepare_in_maps(x, W_qkv, b_qkv, W_out, b_out)
    res = run_bass_kernel_spmd(get_nc(), in_maps, core_ids=list(range(NCORES)))
    acc = np.zeros((B * L, C), np.float32)
    for core_out in res.results:
        acc += core_out["out"].astype(np.float32)
    acc += np.asarray(b_out, np.float32)[None, :]
    return acc.reshape(B, L, C).astype(np.float32)


if __name__ == "__main__":
    rng = np.random.default_rng(0)
    ins = {
        "x": rng.standard_normal((B, L, C)).astype(np.float32),
        "W_qkv": rng.uniform(-1 / 32, 1 / 32, (3 * C, C)).astype(np.float32),
        "b_qkv": rng.uniform(-1 / 32, 1 / 32, (3 * C,)).astype(np.float32),
        "W_out": rng.uniform(-1 / 32, 1 / 32, (C, C)).astype(np.float32),
        "b_out": rng.uniform(-1 / 32, 1 / 32, (C,)).astype(np.float32),
    }
    o = kernel(**ins)
    print(o.shape, o.dtype)


# revision 15
# speedup vs baseline: 1.2836x; 1.2836x over previous
"""Multi-head attention (B=4, L=2048, C=1024, H=16, HD=64) on 8 NeuronCores.

Sharding: tensor-parallel over heads - 2 heads per core. Each core computes
its heads' QKV projection, attention, and a partial output projection over
its 128 ctx channels; the host sums the 8 partial outputs (b_out added on
host). All HW-side matmul operands are bf16 (measured rms rel err 2.4e-3,
well under the 2e-2 gate); PSUM accumulation stays fp32.

Key measured facts driving the layout (TRN2, from ntff traces):
  - PE streams 1 col/cycle @2.4GHz for bf16/f32r/fp8 alike; f32r pays an
    extra 128-cycle self-load per matmul (ldweights can't pipeline fp32),
    so bf16 operands save ~53ns on every 512-col matmul.
  - fp8 DoubleRow only pays off at K>=256 per instruction; scores (K=128)
    and outproj (K=128) get nothing, and fp8 anywhere on the qkv/attnV
    path pushes rms err past 1e-2. So: no fp8.
  - ACT exp [128,1024] from PSUM ~1.1us; exp+identity+copy share one ACT
    table (no swap cost).

Math notes:
  - softmax((q+bq).(k+bk)) == softmax((q+bq).k) per query: the (q+bq).bk
    term is constant across keys. So the k bias is dropped entirely and
    only bq is applied (free, rides the PSUM->SBUF drain).
  - Softmax skips the max-subtraction (scores ~N(0,1/9): exp is safe) and
    normalizes after the ctx matmul using ones-columns in the v tiles
    (rowsum rides as PSUM partition 64 of the ctx accumulation); 1/Z via
    DVE reciprocal + gpsimd partition broadcast (no ACT table swaps).
  - The per-head q chunks are zero-padded to the 128-channel frame (done
    once per tile instance via gpsimd memset) so the scores matmul always
    runs K=128 (PE shape switches measured ~0.9us).
"""

import numpy as np
import ml_dtypes

import concourse.bass as bass
import concourse.mybir as mybir
import concourse.tile as tile
from concourse import bacc
from concourse.bass_utils import run_bass_kernel_spmd

B, L, C, H, HD = 4, 2048, 1024, 16, 64
NCORES = 8
HPC = H // NCORES  # heads per core = 2
F32 = mybir.dt.float32
BF16 = mybir.dt.bfloat16
EXP = mybir.ActivationFunctionType.Exp
LN = mybir.ActivationFunctionType.Ln

LCHUNK = 512          # token chunk for moving operands
NLC = L // LCHUNK     # 4
NKT = L // 128        # 16 k tiles per sequence
NCT = C // 128        # 8 contraction tiles for the projections


def build_kernel():
    nc = bacc.Bacc("TRN2", target_bir_lowering=False, debug=False,
                   num_devices=NCORES)

    xT = nc.dram_tensor("xT", [B, C, L], BF16, kind="ExternalInput")
    # wqkv[j] = [128 c, 1024 (ci,f)]; j in (0=q both heads, 1=k, 2=v)
    wqkv = nc.dram_tensor("wqkv", [3, 128, C], BF16, kind="ExternalInput")
    bqv_d = nc.dram_tensor("bqv", [128, 2], F32, kind="ExternalInput")
    # wo2: [128 c(2 heads), 1024 o]
    wo2 = nc.dram_tensor("wo2", [128, C], BF16, kind="ExternalInput")
    identb_d = nc.dram_tensor("identb_d", [128, 128], BF16, kind="ExternalInput")
    out = nc.dram_tensor("out", [B * L, C], BF16, kind="ExternalOutput")

    with tile.TileContext(nc) as tc:
        kernel_body(nc, tc, xT, wqkv, bqv_d, wo2, identb_d, out)
    nc.compile()
    return nc


def kernel_body(nc, tc, xT, wqkv, bqv_d, wo2, identb_d, out):
    from contextlib import ExitStack
    ctx = ExitStack()
    with ctx:
        consts = ctx.enter_context(tc.tile_pool(name="consts", bufs=1))
        xpool = ctx.enter_context(tc.tile_pool(name="xpool", bufs=16))
        qkvpool = ctx.enter_context(tc.tile_pool(name="qkvpool", bufs=2))
        vppool = ctx.enter_context(tc.tile_pool(name="vppool", bufs=24))
        epool = ctx.enter_context(tc.tile_pool(name="epool", bufs=14))
        cpool = ctx.enter_context(tc.tile_pool(name="cpool", bufs=2))
        spool = ctx.enter_context(tc.tile_pool(name="spool", bufs=2))
        opool = ctx.enter_context(tc.tile_pool(name="opool", bufs=4))
        # PSUM banks: s-tiles 2x2 + cacc 2 + general 2 = 8
        spsum = ctx.enter_context(tc.tile_pool(name="spsum", bufs=2,
                                               space="PSUM"))
        cpsum = ctx.enter_context(tc.tile_pool(name="cpsum", bufs=2,
                                               space="PSUM"))
        gpsum = ctx.enter_context(tc.tile_pool(name="gpsum", bufs=2,
                                               space="PSUM"))

        # ---- constants ----
        # pin the exp+ln ACT table up front: every activation in this kernel
        # (EXP for scores, LN/EXP for 1/Z) lives in act_func_set 6
        # (natural_log_exp_and_others) so no table reloads ever happen.
        nc.scalar.add_instruction(mybir.InstLoadActFuncSet(
            name=nc.get_next_instruction_name(), act_func_set_id=6,
            ins=[], outs=[]))
        wj_tiles = []
        for j in range(3):
            t = consts.tile([128, C], BF16, tag=f"wj{j}", name=f"wj{j}")
            nc.sync.dma_start(out=t, in_=wqkv[j])
            wj_tiles.append(t)
        # w_tiles[ci][j] view into the consolidated per-j weight strips
        w_tiles = [[wj_tiles[j][:, bass.ts(ci, 128)] for j in range(3)]
                   for ci in range(NCT)]
        bqv_t = consts.tile([128, 2], F32, tag="bqv_t")
        nc.sync.dma_start(out=bqv_t, in_=bqv_d[:])
        bq_t = bqv_t[:, 0:1]
        bv_t = bqv_t[:, 1:2]
        identb = consts.tile([128, 128], BF16, tag="identb")
        nc.sync.dma_start(out=identb, in_=identb_d[:])

        # ---- phase helpers (emitted in software-pipelined order below) ----
        def emit_qkv_loads(b, pair):
            # 8 x tiles of [128, 1024] covering token range pair*1024..+1024
            ls = bass.ts(pair, 2 * LCHUNK)
            xts = []
            for ci in range(NCT):
                xt = xpool.tile([128, 2 * LCHUNK], BF16, tag="xt", name="xt")
                nc.sync.dma_start(out=xt, in_=xT[b, bass.ts(ci, 128), ls])
                xts.append(xt)
            return xts

        def emit_qkv_block(b, lc, qkvT, xts):
            ls = bass.ts(lc, LCHUNK)
            xs = bass.ts(lc % 2, LCHUNK)
            for j in range(3):
                p = gpsum.tile([128, LCHUNK], F32, tag="gpb", name="p")
                for ci in range(NCT):
                    nc.tensor.matmul(p, w_tiles[ci][j], xts[ci][:, xs],
                                     start=(ci == 0), stop=(ci == NCT - 1))
                if j == 0:
                    # q: split per head into the zero-padded 128-channel
                    # frame (other head's rows stay zero from the per-tile
                    # memset); bias bq rides the drain.
                    q0p, q1p = qkvT[0]
                    nc.vector.tensor_scalar_add(q0p[0:HD, ls], p[0:HD, :],
                                                bq_t[0:HD])
                    nc.vector.tensor_scalar_add(q1p[HD:128, ls], p[HD:128, :],
                                                bq_t[HD:128])
                elif j == 1:
                    # k: no bias (softmax-invariant, see module docstring)
                    nc.vector.tensor_copy(qkvT[1][:, ls], p)
                else:
                    nc.vector.tensor_scalar_add(qkvT[2][:, ls], p, bv_t)

        def emit_vplus(qkvT):
            # v -> token-major bf16 tiles [128 l, v_h0 | 1 | v_h1 | 1]
            vplus = []
            for t in range(NKT):
                tp = gpsum.tile([128, 128], BF16, tag="gpb", name="tp")
                nc.tensor.transpose(tp, qkvT[2][:, bass.ts(t, 128)], identb[:])
                vp = vppool.tile([128, 2 * HD + 2], BF16, tag="vp", name="vp")
                nc.vector.tensor_copy(vp[:, 0:HD], tp[:, 0:HD])
                nc.vector.tensor_copy(vp[:, HD + 1:2 * HD + 1], tp[:, HD:2 * HD])
                nc.gpsimd.memset(vp[:, HD:HD + 1], 1.0)
                nc.gpsimd.memset(vp[:, 2 * HD + 1:2 * HD + 2], 1.0)
                vplus.append(vp)
            return vplus

        def emit_attn_core(h, qc, qkvT, vplus, ctxT2):
            # one head, one 1024-wide q chunk: scores -> exp -> ctx matmul,
            # then inline normalize (reciprocal + broadcast + fused drain)
            vsl = slice(h * (HD + 1), (h + 1) * (HD + 1))
            q0 = qc * 1024
            caccs = [cpsum.tile([HD + 1, LCHUNK], F32, tag="cpb",
                                name=f"cacc{half}")
                     for half in range(2)]
            evec = []
            qhp = qkvT[0][h]
            for i in range(NKT):
                s = spsum.tile([128, 2 * LCHUNK], F32, tag="spb", name="s")
                for half in range(2):
                    nc.tensor.matmul(
                        s[:, bass.ts(half, LCHUNK)],
                        qkvT[1][:, bass.ts(i, 128)],
                        qhp[:, bass.ds(q0 + half * LCHUNK, LCHUNK)],
                        start=True, stop=True)
                e = epool.tile([128, 2 * LCHUNK], BF16, tag="e", name="e")
                nc.scalar.activation(e, s, EXP, scale=0.125)
                evec.append(e)
            for i in range(NKT):
                for half in range(2):
                    nc.tensor.matmul(
                        caccs[half],
                        vplus[i][:, vsl],
                        evec[i][:, bass.ts(half, LCHUNK)],
                        start=(i == 0), stop=(i == NKT - 1))
            for half in range(2):
                # 1/Z = exp(-ln Z) on ACT (both funcs live in the pinned
                # table; hw divide/reciprocal are slow or unsupported)
                cacc = caccs[half]
                qs = bass.ds(q0 + half * LCHUNK, LCHUNK)
                lnz = spool.tile([1, LCHUNK], F32, tag="lnz", name="lnz", bufs=4)
                nc.scalar.activation(lnz[0:1, :], cacc[HD:HD + 1, :], LN)
                zr = spool.tile([1, LCHUNK], F32, tag="zr", name="zr", bufs=4)
                nc.scalar.activation(zr[0:1, :], lnz[0:1, :], EXP, scale=-1.0)
                zs = spool.tile([HD, LCHUNK], F32, tag="zs", name="zs", bufs=3)
                nc.gpsimd.partition_broadcast(zs[0:HD, :], zr[0:1, :])
                nc.vector.tensor_mul(ctxT2[h * HD:h * HD + HD, qs],
                                     cacc[0:HD, :], zs)

        def emit_outproj(b, ctxT2, trange, act_share=False):
            # act_share: route half the PSUM drains through ACT Copy (only
            # used for the final tiles, when the exp stream has gone idle)
            for t in trange:
                rows = bass.ds(b * L + t * 128, 128)
                ot = opool.tile([128, C], BF16, tag="ot", name="ot")
                for oc in range(C // 512):
                    os_ = bass.ts(oc, 512)
                    o = gpsum.tile([128, 512], F32, tag="gpb", name="o")
                    nc.tensor.matmul(o, ctxT2[:, bass.ts(t, 128)],
                                     wo_t[:, os_], start=True, stop=True)
                    if act_share and oc == 1:
                        nc.scalar.copy(ot[:, os_], o)
                    else:
                        nc.vector.tensor_copy(ot[:, os_], o)
                nc.sync.dma_start(out=out[rows, :], in_=ot)

        # ---- software-pipelined emission ----
        # Interleave next batch's qkv blocks between attention chunks so the
        # PE's in-order queue always has dense, ready work behind any stall.
        def new_qkvT():
            qp = tuple(qkvpool.tile([128, L], BF16, tag=f"q{h}p",
                                    name=f"q{h}p") for h in range(HPC))
            kc = qkvpool.tile([128, L], BF16, tag="kc", name="kc")
            vc = qkvpool.tile([128, L], BF16, tag="vc", name="vc")
            # zero the other head's rows once per tile instance; the q
            # drains only write their own head's 64 rows.
            nc.gpsimd.memset(qp[0][HD:128, :], 0.0)
            nc.gpsimd.memset(qp[1][0:HD, :], 0.0)
            return [qp, kc, vc]

        qkvT = new_qkvT()
        loads = {0: emit_qkv_loads(0, 0), 1: emit_qkv_loads(0, 1)}
        # wo is first needed by outproj, ~1 chunk into batch 0: load it
        # after the prologue x tiles so it doesn't delay the first matmul
        wo_t = consts.tile([128, C], BF16, tag="wo_t")
        nc.sync.dma_start(out=wo_t, in_=wo2[:])
        for lc in range(NLC):
            emit_qkv_block(0, lc, qkvT, loads[lc // 2])
        vplus = emit_vplus(qkvT)
        for b in range(B):
            ctxT2 = cpool.tile([128, L], BF16, tag="ctxT2", name="ctxT2")
            nxt = new_qkvT() if b + 1 < B else None
            # column-half order: both heads of q-chunk 0, then q-chunk 1, so
            # outproj l-tiles can start after the first half completes
            chunks = [(0, 0), (1, 0), (0, 1), (1, 1)]
            loads = {}
            if nxt is not None:
                loads[0] = emit_qkv_loads(b + 1, 0)
            for k, (h, qc) in enumerate(chunks):
                emit_attn_core(h, qc, qkvT, vplus, ctxT2)
                if nxt is not None:
                    if k == 1:
                        loads[1] = emit_qkv_loads(b + 1, 1)
                    emit_qkv_block(b + 1, k, nxt, loads[k // 2])
                if k == 1:
                    emit_outproj(b, ctxT2, range(0, NKT // 2))
            if nxt is not None:
                nxt_vplus = emit_vplus(nxt)
            emit_outproj(b, ctxT2, range(NKT // 2, NKT),
                         act_share=(nxt is None))
            if nxt is not None:
                qkvT, vplus = nxt, nxt_vplus


_NC_CACHE = None


def get_nc():
    global _NC_CACHE
    if _NC_CACHE is None:
        _NC_CACHE = build_kernel()
    return _NC_CACHE


def prepare_in_maps(x, W_qkv, b_qkv, W_out, b_out):
    x = np.asarray(x, np.float32)
    W_qkv = np.asarray(W_qkv, np.float32)
    b_qkv = np.asarray(b_qkv, np.float32)
    W_out = np.asarray(W_out, np.float32)

    xT = np.ascontiguousarray(x.transpose(0, 2, 1)).astype(ml_dtypes.bfloat16)

    in_maps = []
    for core in range(NCORES):
        h0 = HPC * core
        # per-head channel rows in W_qkv: q = h*192..+64, k = +64, v = +128
        qrows = [np.arange(h * 192, h * 192 + 64) for h in (h0, h0 + 1)]
        krows = [q + 64 for q in qrows]
        vrows = [q + 128 for q in qrows]
        fq = np.concatenate(qrows)
        fk = np.concatenate(krows)
        fv = np.concatenate(vrows)
        # wqkv strips: [j, 128 c, 1024 (ci,f)]
        wt = np.empty((3, 128, C), ml_dtypes.bfloat16)
        for j, rows in enumerate((fq, fk, fv)):
            wT = np.ascontiguousarray(W_qkv[rows].T)  # [1024 c, 128 f]
            # [ci, 128 c, 128 f] -> [128 c, ci*128 + f]
            wt[j] = wT.reshape(NCT, 128, 128).transpose(1, 0, 2).reshape(
                128, C).astype(ml_dtypes.bfloat16)
        # wo2 = [128 c, 1024 o]: rows 0:64 h0 ctx channels, 64:128 h1
        wo2 = np.concatenate([
            np.ascontiguousarray(W_out[:, (h0 + h) * HD:(h0 + h + 1) * HD].T)
            for h in range(HPC)
        ], axis=0)
        in_maps.append({
            "xT": xT,
            "wqkv": wt,
            "bqv": np.ascontiguousarray(
                np.stack([b_qkv[fq], b_qkv[fv]], axis=1), np.float32),
            "wo2": np.ascontiguousarray(wo2).astype(ml_dtypes.bfloat16),
            "identb_d": np.eye(128, dtype=ml_dtypes.bfloat16),
        })
    return in_maps


def kernel(x, W_qkv, b_qkv, W_out, b_out):
    in_maps = prepare_in_maps(x, W_qkv, b_qkv, W_out, b_out)
    res = run_bass_kernel_spmd(get_nc(), in_maps, core_ids=list(range(NCORES)))
    acc = np.zeros((B * L, C), np.float32)
    for core_out in res.results:
        acc += core_out["out"].astype(np.float32)
    acc += np.asarray(b_out, np.float32)[None, :]
    return acc.reshape(B, L, C).astype(np.float32)


if __name__ == "__main__":
    rng = np.random.default_rng(0)
    ins = {
        "x": rng.standard_normal((B, L, C)).astype(np.float32),
        "W_qkv": rng.uniform(-1 / 32, 1 / 32, (3 * C, C)).astype(np.float32),
        "b_qkv": rng.uniform(-1 / 32, 1 / 32, (3 * C,)).astype(np.float32),
        "W_out": rng.uniform(-1 / 32, 1 / 32, (C, C)).astype(np.float32),
        "b_out": rng.uniform(-1 / 32, 1 / 32, (C,)).astype(np.float32),
    }
    o = kernel(**ins)
    print(o.shape, o.dtype)
